# revision 10
# baseline (speedup 1.0000x reference)
"""Trainium2 Bass kernel for nn_MultiHeadAttention_49486613184863.

Structure exploited: the reference reshapes (B, S, 3*dk*H) -> (B, H, S, 3*dk)
with NO transpose, so head h of batch b only consumes x[b, 128h:128(h+1), :],
and its 2048-long "attention sequence" enumerates (position-in-block, group)
pairs.  We reorder the kv sequence group-major (softmax is permutation
invariant when k and v are permuted together) which makes every on-chip
operand a clean access pattern.

Sharding: 8 cores, core c -> (batch b = c//2, query-half qh = c%2).  The host
permutes each 128-row block of x so the core's query half sits first; the
program is identical on all cores (SPMD).  No cross-core communication.

Per-core program (all matmuls bf16 -> fp32 PSUM):
  1. QKV projection, transposed layouts (weights pre-transposed on host).
     Bias folded in via an appended ones-column on x (only when biases != 0).
     V gets an extra "ones" column per group -> attention row-sums fall out
     of the attn@v matmul for free.
  2. Per head: scores^T = k @ q^T (K=64 matmuls pair-packed on PE row-groups
     0/64), exp on ScalarE straight out of PSUM (scale=1/8 fused; no max
     subtraction -- scores are ~N(0, 0.25)), p^T @ v via K=128 matmuls
     accumulating o^T[65, q'] in PSUM, then normalize by the sum row
     (DVE reciprocal + GpSimd partition_broadcast + DVE multiply).
  3. o_proj as K=64 accumulation over heads; host re-permutes rows and adds b_o.
"""

import sys

for p in ("/opt/trn_rl_repo", "/opt/pypackages"):
    if p not in sys.path:
        sys.path.insert(0, p)

import numpy as np
import ml_dtypes

import concourse.bass as bass
import concourse.mybir as mybir
import concourse.tile as tile
from concourse.tile import ScopedClock
from concourse.bass_utils import run_bass_kernel_spmd

# ---------------------------------------------------------------------------
# Workaround: this walrus build rejects instructions carrying more than one
# sem wait ("Too many sync wait commands").  After Tile scheduling, move the
# excess waits of every instruction onto preceding same-engine nops --
# semantics are identical (the engine blocks at the nop instead).
# ---------------------------------------------------------------------------
_MAX_WAITS = 1


def _split_sync_waits(nc: bass.Bass):
    n = 0
    for bbh in nc.bb_map.values():
        bb = bbh.bb
        new = []
        for inst in bb.instructions:
            si = inst.sync_info
            waits = list(si.on_wait) if si is not None and si.on_wait else []
            if len(waits) > _MAX_WAITS:
                extra, keep = waits[:-_MAX_WAITS], waits[-_MAX_WAITS:]
                for i in range(0, len(extra), _MAX_WAITS):
                    nop = mybir.InstNoOp(
                        name=f"{inst.name}-wsplit{n}", ins=[], outs=[])
                    n += 1
                    nop.engine = inst.engine
                    nop.sync_info = mybir.SyncInfo(
                        on_wait=extra[i:i + _MAX_WAITS], on_update=[])
                    nc.register_instruction(nop)
                    new.append(nop)
                si.on_wait = keep
            new.append(inst)
        bb.instructions = new


# ---------------------------------------------------------------------------

N_CORES = 8
H = 16          # heads == groups
DK = 64
D = 1024        # input dim == dk*H
S = 2048        # sequence length
B = 4           # batch
P = 128
BF16 = mybir.dt.bfloat16
F32 = mybir.dt.float32
Exp = mybir.ActivationFunctionType.Exp

_PROGRAM_CACHE: dict = {}


def _build_program(with_bias: bool) -> bass.Bass:
    C = D + 1 if with_bias else D          # contraction dim of projections
    NK = (C + P - 1) // P                  # 9 when bias row present, else 8
    VW = H * 65                            # 1040: v features + ones col per group

    nc = bass.Bass("TRN2", target_bir_lowering=False, debug=False,
                   num_devices=N_CORES)

    xT = nc.declare_dram_parameter("xT", [C, S], BF16, isOutput=False)
    wqk = nc.declare_dram_parameter("wqk", [C, 2 * D], BF16, isOutput=False)
    wv = nc.declare_dram_parameter("wv", [C, VW], BF16, isOutput=False)
    wo = nc.declare_dram_parameter("wo", [D, D], BF16, isOutput=False)
    out = nc.declare_dram_parameter("out", [D, D], F32, isOutput=True)

    def ksz(k):  # rows in contraction k-tile k
        return min(P, C - P * k)

    with tile.TileContext(nc) as tc:
        with (
            tc.tile_pool(name="resq", bufs=1) as resq,
            tc.tile_pool(name="resk", bufs=1) as resk,
            tc.tile_pool(name="resv", bufs=1) as resv,
            tc.tile_pool(name="resoc", bufs=1) as resoc,
        ):
            # ---- persistent SBUF tensors ----
            QT = resq.tile([P, H * 1024], BF16, tag="qt")   # [f+64*(g%2) dup, (h,g,p<64)]
            KT = resk.tile([P, 8 * S], BF16, tag="kt")      # [f+64*(g%2), (gp, 128h+p)]
            V = resv.tile([P, H * VW], BF16, tag="v")       # [pos_p, (h, 65g+f)]
            OC = [resoc.tile([64, 1024], BF16, tag=f"oc{h}", name=f"oc{h}")
                  for h in range(H)]

            QTv = QT[:].rearrange("f (h g p) -> f h g p", h=H, g=H)

            # ================= phase 1: projections =================
            with (
                tc.tile_pool(name="xin", bufs=1) as xin,
                tc.tile_pool(name="wvin", bufs=1) as wvin,
                tc.tile_pool(name="wqkin", bufs=4) as wqkin,
                tc.tile_pool(name="pps", bufs=3, space="PSUM") as pps,
            ):
                xsb = []
                for k in range(NK):
                    t = xin.tile([ksz(k), S], BF16, tag=f"x{k}")
                    nc.sync.dma_start(t[:], xT[P * k:P * k + ksz(k), :])
                    xsb.append(t)
                wvsb = []
                for k in range(NK):
                    t = wvin.tile([ksz(k), VW], BF16, tag=f"wv{k}")
                    nc.sync.dma_start(t[:], wv[P * k:P * k + ksz(k), :])
                    wvsb.append(t)

                # --- Q rows (wqk cols 128t..): out free = (h, p<64) ---
                for t in range(8):
                    for n in range(2):          # (h,p) slices of 512
                        ps = pps.tile([P, 512], F32, tag="pp")
                        for k in range(NK):
                            wt = wqkin.tile([ksz(k), P], BF16, tag="wq")
                            nc.sync.dma_start(
                                wt[:], wqk[P * k:P * k + ksz(k), P * t:P * t + P])
                            xv = xsb[k][:].rearrange("c (h p) -> c h p", h=H)
                            nc.tensor.matmul(
                                ps[:], wt[:], xv[:, 8 * n:8 * n + 8, 0:64],
                                start=(k == 0), stop=(k == NK - 1))
                        # rows 0:64 -> g=2t (copy A), rows 64:128 -> g=2t+1 (copy B)
                        psv = ps[:].rearrange("f (h p) -> f h p", h=8)
                        nc.vector.tensor_copy(
                            QTv[0:64, 8 * n:8 * n + 8, 2 * t, :], psv[0:64])
                        nc.vector.tensor_copy(
                            QTv[64:128, 8 * n:8 * n + 8, 2 * t + 1, :], psv[64:128])

                # --- K rows (wqk cols 1024 + 128t..): out free = all pos ---
                for t in range(8):
                    for n in range(4):
                        ps = pps.tile([P, 512], F32, tag="pp")
                        for k in range(NK):
                            wt = wqkin.tile([ksz(k), P], BF16, tag="wq")
                            nc.sync.dma_start(
                                wt[:],
                                wqk[P * k:P * k + ksz(k), D + P * t:D + P * t + P])
                            nc.tensor.matmul(
                                ps[:], wt[:], xsb[k][:, 512 * n:512 * n + 512],
                                start=(k == 0), stop=(k == NK - 1))
                        nc.vector.tensor_copy(
                            KT[:, S * t + 512 * n: S * t + 512 * n + 512], ps[:])

                # --- V: out [pos-block h, vfeat] ---
                for h in range(H):
                    for lo, hi in ((0, 512), (512, 1024), (1024, VW)):
                        ps = pps.tile([P, 512], F32, tag="pp")
                        for k in range(NK):
                            nc.tensor.matmul(
                                ps[:, 0:hi - lo],
                                xsb[k][:, P * h:P * h + P],
                                wvsb[k][:, lo:hi],
                                start=(k == 0), stop=(k == NK - 1))
                        nc.vector.tensor_copy(
                            V[:, VW * h + lo: VW * h + hi], ps[:, 0:hi - lo])

                if not with_bias:
                    # ones columns (65g+64) built by memset instead of matmul
                    Vv = V[:].rearrange("p (h g f) -> p h g f", h=H, g=H)
                    nc.vector.memset(Vv[:, :, :, 64:65], 1.0)

                # QT cross-fill: copy A holds even g, copy B odd g; DMA the
                # missing halves across partitions (SBUF->SBUF).
                nc.sync.dma_start(QTv[64:128, :, 0:16:2, :], QTv[0:64, :, 0:16:2, :])
                nc.sync.dma_start(QTv[0:64, :, 1:16:2, :], QTv[64:128, :, 1:16:2, :])

            # ================= phase 2: attention =================
            with (
                tc.tile_pool(name="scps", bufs=2, space="PSUM") as scps,
                tc.tile_pool(name="ops", bufs=3, space="PSUM") as ops,
                tc.tile_pool(name="rbps", bufs=1, space="PSUM") as rbps,
                tc.tile_pool(name="pt", bufs=6) as ptp,
                tc.tile_pool(name="rr", bufs=4) as rrp,
                tc.tile_pool(name="onesp", bufs=1) as onesp,
            ):
                ones64 = onesp.tile([1, 64], BF16, tag="ones64")
                nc.vector.memset(ones64[:], 1.0)
                for h in range(H):
                    po = [ops.tile([65, 512], F32, tag="po", name=f"po{h}_{j}")
                          for j in range(2)]
                    for gp in range(8):
                        for j in range(2):
                            sc = scps.tile([P, 1024], F32, tag="sc")
                            for par in range(2):            # g_k = 2gp+par
                                lo, hi = 64 * par, 64 * par + 64
                                nc.tensor.matmul(
                                    sc[:, 512 * par:512 * par + 512],
                                    KT[lo:hi, S * gp + P * h: S * gp + P * h + P],
                                    QT[lo:hi, 1024 * h + 512 * j:
                                       1024 * h + 512 * j + 512],
                                    start=True, stop=True)
                            pt = ptp.tile([P, 1024], BF16, tag="pt")
                            nc.scalar.activation(pt[:], sc[:], Exp,
                                                 bias=0.0, scale=0.125)
                            for par in range(2):
                                g = 2 * gp + par
                                nc.tensor.matmul(
                                    po[j][:],
                                    V[:, VW * h + 65 * g: VW * h + 65 * g + 65],
                                    pt[:, 512 * par:512 * par + 512],
                                    start=(gp == 0 and par == 0),
                                    stop=(gp == 7 and par == 1))
                    for j in range(2):
                        r = rrp.tile([1, 512], F32, tag="r")
                        nc.vector.reciprocal(r[:], po[j][64:65, :])
                        r16 = rrp.tile([1, 512], BF16, tag="r16")
                        nc.vector.tensor_copy(r16[:], r[:])
                        pb = rbps.tile([64, 512], F32, tag="pb")
                        nc.tensor.matmul(pb[:], ones64[:], r16[:],
                                         start=True, stop=True)
                        rb = rrp.tile([64, 512], F32, tag="rb")
                        nc.vector.tensor_copy(rb[:], pb[:])
                        nc.vector.tensor_mul(
                            OC[h][:, 512 * j:512 * j + 512], po[j][0:64, :], rb[:])

            # ================= phase 3: o_proj =================
            with (
                tc.tile_pool(name="wos", bufs=1) as wos,
                tc.tile_pool(name="oout", bufs=3) as oout,
                tc.tile_pool(name="ops", bufs=2, space="PSUM") as opps,
            ):
                wosb = []
                for h in range(H):
                    t = wos.tile([64, D], BF16, tag=f"wo{h}")
                    nc.sync.dma_start(t[:], wo[64 * h:64 * h + 64, :])
                    wosb.append(t)
                for m in range(8):
                    for n in range(2):
                        ps = opps.tile([P, 512], F32, tag="op")
                        for h in range(H):
                            nc.tensor.matmul(
                                ps[:], OC[h][:, P * m:P * m + P],
                                wosb[h][:, 512 * n:512 * n + 512],
                                start=(h == 0), stop=(h == H - 1))
                        ot = oout.tile([P, 512], F32, tag="ot")
                        nc.scalar.copy(ot[:], ps[:])
                        nc.sync.dma_start(
                            out[P * m:P * m + P, 512 * n:512 * n + 512], ot[:])

    _split_sync_waits(nc)
    return nc


def _prep_host(x, W_qkv, b_qkv, W_o, with_bias):
    """Build per-core input maps (bf16, pre-transposed/permuted)."""
    bf = ml_dtypes.bfloat16
    Wr = W_qkv.reshape(H, 3 * DK, D)
    br = b_qkv.reshape(H, 3 * DK)

    # wqk: [C, 2048]; cols: r<1024 -> q feat (64g+f), else k feat
    W_qk = np.concatenate(
        [Wr[:, 0:64, :].reshape(H * 64, D), Wr[:, 64:128, :].reshape(H * 64, D)],
        axis=0)
    wqk = W_qk.T
    # wv: [C, 1040]; col 65g+f (f<64) = v feat, col 65g+64 = ones indicator
    C = D + 1 if with_bias else D
    wv = np.zeros((C, H * 65), dtype=np.float32)
    for g in range(H):
        wv[:D, 65 * g:65 * g + 64] = Wr[g, 128:, :].T
        if with_bias:
            wv[D, 65 * g:65 * g + 64] = br[g, 128:]
            wv[D, 65 * g + 64] = 1.0
    if with_bias:
        b_qk = np.concatenate([br[:, 0:64].reshape(-1), br[:, 64:128].reshape(-1)])
        wqk = np.concatenate([wqk, b_qk[None, :]], axis=0)
    wqk = np.ascontiguousarray(wqk, dtype=np.float32).astype(bf)
    wv = wv.astype(bf)
    wo = np.ascontiguousarray(W_o.T, dtype=np.float32).astype(bf)

    in_maps = []
    for c in range(N_CORES):
        b, qh = divmod(c, 2)
        xb = x[b].reshape(H, 128, D)
        if qh == 0:
            xp = xb
        else:
            xp = np.concatenate([xb[:, 64:, :], xb[:, :64, :]], axis=1)
        xp = xp.reshape(S, D).T                      # [1024, 2048]
        if with_bias:
            xp = np.concatenate([xp, np.ones((1, S), np.float32)], axis=0)
        in_maps.append({
            "xT": np.ascontiguousarray(xp, dtype=np.float32).astype(bf),
            "wqk": wqk, "wv": wv, "wo": wo,
        })
    return in_maps


def kernel(x, W_qkv, b_qkv, W_o, b_o):
    x = np.asarray(x, dtype=np.float32)
    W_qkv = np.asarray(W_qkv, dtype=np.float32)
    b_qkv = np.asarray(b_qkv, dtype=np.float32)
    W_o = np.asarray(W_o, dtype=np.float32)
    b_o = np.asarray(b_o, dtype=np.float32)

    with_bias = bool(np.any(b_qkv != 0.0))
    if with_bias not in _PROGRAM_CACHE:
        _PROGRAM_CACHE[with_bias] = _build_program(with_bias)
    nc = _PROGRAM_CACHE[with_bias]

    in_maps = _prep_host(x, W_qkv, b_qkv, W_o, with_bias)
    res = run_bass_kernel_spmd(nc, in_maps, list(range(N_CORES)))

    out = np.empty((B, S, D), dtype=np.float32)
    for c in range(N_CORES):
        b, qh = divmod(c, 2)
        oc = res.results[c]["out"]                   # rows q' = 64g + p
        out[b, 1024 * qh:1024 * qh + 1024, :] = (
            oc.reshape(H, 64, D).transpose(1, 0, 2).reshape(1024, D))
    out += b_o
    return out


if __name__ == "__main__":
    rng = np.random.default_rng(0)
    inputs = {
        "x": rng.standard_normal((B, S, D)).astype(np.float32),
        "W_qkv": (rng.standard_normal((3 * DK * H, D)) * 0.04).astype(np.float32),
        "b_qkv": np.zeros((3 * DK * H,), np.float32),
        "W_o": (rng.standard_normal((DK * H, DK * H)) * 0.03).astype(np.float32),
        "b_o": np.zeros((DK * H,), np.float32),
    }
    o = kernel(**inputs)
    print("kernel out", o.shape, o.dtype)


# revision 13
# speedup vs baseline: 1.3499x; 1.3499x over previous
"""Trainium2 Bass kernel for nn_MultiHeadAttention_49486613184863.

Structure exploited: the reference reshapes (B, S, 3*dk*H) -> (B, H, S, 3*dk)
with NO transpose, so head h of batch b only consumes x[b, 128h:128(h+1), :],
and its 2048-long "attention sequence" enumerates (position-in-block, group)
pairs.  We reorder the kv sequence group-major (softmax is permutation
invariant when k and v are permuted together) which makes every on-chip
operand a clean access pattern.

Sharding: 8 cores, core c -> (batch b = c//2, query-half qh = c%2).  The host
permutes each 128-row block of x so the core's query half sits first; the
program is identical on all cores (SPMD).  No cross-core communication.

Per-core program (all matmuls bf16 -> fp32 PSUM):
  1. QKV projection, transposed layouts (weights pre-transposed on host).
     Bias folded in via an appended ones-column on x (only when biases != 0).
     V gets an extra "ones" column per group -> attention row-sums fall out
     of the attn@v matmul for free.
  2. Per head: scores^T = k @ q^T (K=64 matmuls pair-packed on PE row-groups
     0/64), exp on ScalarE straight out of PSUM (scale=1/8 fused; no max
     subtraction -- scores are ~N(0, 0.25)), p^T @ v via K=128 matmuls
     accumulating o^T[65, q'] in PSUM, then normalize by the sum row
     (DVE reciprocal + GpSimd partition_broadcast + DVE multiply).
  3. o_proj as K=64 accumulation over heads; host re-permutes rows and adds b_o.
"""

import sys

for p in ("/opt/trn_rl_repo", "/opt/pypackages"):
    if p not in sys.path:
        sys.path.insert(0, p)

import numpy as np
import ml_dtypes

import concourse.bass as bass
import concourse.mybir as mybir
import concourse.tile as tile
from concourse.tile import ScopedClock
from concourse.bass_utils import run_bass_kernel_spmd

# ---------------------------------------------------------------------------
# Workaround: this walrus build rejects instructions carrying more than one
# sem wait ("Too many sync wait commands").  After Tile scheduling, move the
# excess waits of every instruction onto preceding same-engine nops --
# semantics are identical (the engine blocks at the nop instead).
# ---------------------------------------------------------------------------
_MAX_WAITS = 1


def _split_sync_waits(nc: bass.Bass):
    n = 0
    for bbh in nc.bb_map.values():
        bb = bbh.bb
        new = []
        for inst in bb.instructions:
            si = inst.sync_info
            waits = list(si.on_wait) if si is not None and si.on_wait else []
            if len(waits) > _MAX_WAITS:
                extra, keep = waits[:-_MAX_WAITS], waits[-_MAX_WAITS:]
                for i in range(0, len(extra), _MAX_WAITS):
                    nop = mybir.InstNoOp(
                        name=f"{inst.name}-wsplit{n}", ins=[], outs=[])
                    n += 1
                    nop.engine = inst.engine
                    nop.sync_info = mybir.SyncInfo(
                        on_wait=extra[i:i + _MAX_WAITS], on_update=[])
                    nc.register_instruction(nop)
                    new.append(nop)
                si.on_wait = keep
            new.append(inst)
        bb.instructions = new


# ---------------------------------------------------------------------------

N_CORES = 8
H = 16          # heads == groups
DK = 64
D = 1024        # input dim == dk*H
S = 2048        # sequence length
B = 4           # batch
P = 128
BF16 = mybir.dt.bfloat16
F32 = mybir.dt.float32
Exp = mybir.ActivationFunctionType.Exp

_PROGRAM_CACHE: dict = {}


def _build_program(with_bias: bool) -> bass.Bass:
    C = D + 1 if with_bias else D          # contraction dim of projections
    NK = (C + P - 1) // P                  # 9 when bias row present, else 8
    VW = H * 65                            # 1040: v features + ones col per group

    nc = bass.Bass("TRN2", target_bir_lowering=False, debug=False,
                   num_devices=N_CORES)

    xT = nc.declare_dram_parameter("xT", [C, S], BF16, isOutput=False)
    wqk = nc.declare_dram_parameter("wqk", [C, 2 * D], BF16, isOutput=False)
    wv = nc.declare_dram_parameter("wv", [C, VW], BF16, isOutput=False)
    wo = nc.declare_dram_parameter("wo", [D, D], BF16, isOutput=False)
    out = nc.declare_dram_parameter("out", [D, D], F32, isOutput=True)

    def ksz(k):  # rows in contraction k-tile k
        return min(P, C - P * k)

    with tile.TileContext(nc) as tc:
        with (
            tc.tile_pool(name="resq", bufs=1) as resq,
            tc.tile_pool(name="resk", bufs=1) as resk,
            tc.tile_pool(name="resv", bufs=1) as resv,
            tc.tile_pool(name="resoc", bufs=1) as resoc,
        ):
            # ---- persistent SBUF tensors ----
            QT = resq.tile([P, H * 1024], BF16, tag="qt")   # [f+64*(g%2) dup, (h,g,p<64)]
            KT = resk.tile([P, 8 * S], BF16, tag="kt")      # [f+64*(g%2), (gp, 128h+p)]
            V = resv.tile([P, H * VW], BF16, tag="v")       # [pos_p, (h, 65g+f)]
            OC = [resoc.tile([64, 1024], BF16, tag=f"oc{h}", name=f"oc{h}")
                  for h in range(H)]

            QTv = QT[:].rearrange("f (h g p) -> f h g p", h=H, g=H)

            # ================= phase 1: projections =================
            with (
                tc.tile_pool(name="xin", bufs=1) as xin,
                tc.tile_pool(name="wvin", bufs=1) as wvin,
                tc.tile_pool(name="wqkin", bufs=2) as wqkin,
                tc.tile_pool(name="pps", bufs=3, space="PSUM") as pps,
            ):
                xsb = []
                for k in range(NK):
                    t = xin.tile([ksz(k), S], BF16, tag=f"x{k}")
                    nc.sync.dma_start(t[:], xT[P * k:P * k + ksz(k), :])
                    xsb.append(t)
                wvsb = []
                for k in range(NK):
                    t = wvin.tile([ksz(k), VW], BF16, tag=f"wv{k}")
                    nc.sync.dma_start(t[:], wv[P * k:P * k + ksz(k), :])
                    wvsb.append(t)

                # --- Q rows (wqk cols 128t..): out free = (h, p<64) ---
                for t in range(8):
                    wts = []
                    for k in range(NK):
                        wt = wqkin.tile([ksz(k), P], BF16, tag=f"wq{k}",
                                        name=f"wq{t}_{k}")
                        nc.sync.dma_start(
                            wt[:], wqk[P * k:P * k + ksz(k), P * t:P * t + P])
                        wts.append(wt)
                    for n in range(2):          # (h,p) slices of 512
                        ps = pps.tile([P, 512], F32, tag="pp")
                        for k in range(NK):
                            xv = xsb[k][:].rearrange("c (h p) -> c h p", h=H)
                            nc.tensor.matmul(
                                ps[:], wts[k][:], xv[:, 8 * n:8 * n + 8, 0:64],
                                start=(k == 0), stop=(k == NK - 1))
                        # rows 0:64 -> g=2t (copy A), rows 64:128 -> g=2t+1 (copy B)
                        psv = ps[:].rearrange("f (h p) -> f h p", h=8)
                        nc.vector.tensor_copy(
                            QTv[0:64, 8 * n:8 * n + 8, 2 * t, :], psv[0:64])
                        nc.vector.tensor_copy(
                            QTv[64:128, 8 * n:8 * n + 8, 2 * t + 1, :], psv[64:128])

                # --- K rows (wqk cols 1024 + 128t..): out free = all pos ---
                for t in range(8):
                    wts = []
                    for k in range(NK):
                        wt = wqkin.tile([ksz(k), P], BF16, tag=f"wq{k}",
                                        name=f"wk{t}_{k}")
                        nc.sync.dma_start(
                            wt[:],
                            wqk[P * k:P * k + ksz(k), D + P * t:D + P * t + P])
                        wts.append(wt)
                    for n in range(4):
                        ps = pps.tile([P, 512], F32, tag="pp")
                        for k in range(NK):
                            nc.tensor.matmul(
                                ps[:], wts[k][:], xsb[k][:, 512 * n:512 * n + 512],
                                start=(k == 0), stop=(k == NK - 1))
                        nc.vector.tensor_copy(
                            KT[:, S * t + 512 * n: S * t + 512 * n + 512], ps[:])

                # --- V: out [pos-block h, vfeat] ---
                for h in range(H):
                    for lo, hi in ((0, 512), (512, 1024), (1024, VW)):
                        ps = pps.tile([P, 512], F32, tag="pp")
                        for k in range(NK):
                            nc.tensor.matmul(
                                ps[:, 0:hi - lo],
                                xsb[k][:, P * h:P * h + P],
                                wvsb[k][:, lo:hi],
                                start=(k == 0), stop=(k == NK - 1))
                        nc.vector.tensor_copy(
                            V[:, VW * h + lo: VW * h + hi], ps[:, 0:hi - lo])

                if not with_bias:
                    # ones columns (65g+64) built by memset instead of matmul
                    Vv = V[:].rearrange("p (h g f) -> p h g f", h=H, g=H)
                    nc.vector.memset(Vv[:, :, :, 64:65], 1.0)

                # QT cross-fill: copy A holds even g, copy B odd g; DMA the
                # missing halves across partitions (SBUF->SBUF).
                nc.sync.dma_start(QTv[64:128, :, 0:16:2, :], QTv[0:64, :, 0:16:2, :])
                nc.sync.dma_start(QTv[0:64, :, 1:16:2, :], QTv[64:128, :, 1:16:2, :])

            # ================= phase 2: attention =================
            with (
                tc.tile_pool(name="scps", bufs=2, space="PSUM") as scps,
                tc.tile_pool(name="ops", bufs=3, space="PSUM") as ops,
                tc.tile_pool(name="rbps", bufs=1, space="PSUM") as rbps,
                tc.tile_pool(name="pt", bufs=6) as ptp,
                tc.tile_pool(name="rr", bufs=4) as rrp,
                tc.tile_pool(name="onesp", bufs=1) as onesp,
            ):
                ones64 = onesp.tile([1, 64], BF16, tag="ones64")
                nc.vector.memset(ones64[:], 1.0)
                for h in range(H):
                    po = [ops.tile([65, 512], F32, tag="po", name=f"po{h}_{j}")
                          for j in range(2)]
                    for gp in range(8):
                        for j in range(2):
                            sc = scps.tile([P, 1024], F32, tag="sc")
                            for par in range(2):            # g_k = 2gp+par
                                lo, hi = 64 * par, 64 * par + 64
                                nc.tensor.matmul(
                                    sc[:, 512 * par:512 * par + 512],
                                    KT[lo:hi, S * gp + P * h: S * gp + P * h + P],
                                    QT[lo:hi, 1024 * h + 512 * j:
                                       1024 * h + 512 * j + 512],
                                    start=True, stop=True)
                            pt = ptp.tile([P, 1024], BF16, tag="pt")
                            nc.scalar.activation(pt[:], sc[:], Exp,
                                                 bias=0.0, scale=0.125)
                            for par in range(2):
                                g = 2 * gp + par
                                nc.tensor.matmul(
                                    po[j][:],
                                    V[:, VW * h + 65 * g: VW * h + 65 * g + 65],
                                    pt[:, 512 * par:512 * par + 512],
                                    start=(gp == 0 and par == 0),
                                    stop=(gp == 7 and par == 1))
                    for j in range(2):
                        # broadcast the sums row via PE, then reciprocal on
                        # all 64 lanes (a [1,512] DVE op runs on one lane)
                        s16 = rrp.tile([1, 512], BF16, tag="s16")
                        nc.vector.tensor_copy(s16[:], po[j][64:65, :])
                        pb = rbps.tile([64, 512], F32, tag="pb")
                        nc.tensor.matmul(pb[:], ones64[:], s16[:],
                                         start=True, stop=True)
                        rb = rrp.tile([64, 512], F32, tag="rb")
                        nc.vector.reciprocal(rb[:], pb[:])
                        nc.vector.tensor_mul(
                            OC[h][:, 512 * j:512 * j + 512], po[j][0:64, :], rb[:])

            # ================= phase 3: o_proj =================
            with (
                tc.tile_pool(name="wos", bufs=1) as wos,
                tc.tile_pool(name="oout", bufs=3) as oout,
                tc.tile_pool(name="ops", bufs=2, space="PSUM") as opps,
            ):
                wosb = []
                for h in range(H):
                    t = wos.tile([64, D], BF16, tag=f"wo{h}")
                    nc.sync.dma_start(t[:], wo[64 * h:64 * h + 64, :])
                    wosb.append(t)
                for m in range(8):
                    for n in range(2):
                        ps = opps.tile([P, 512], F32, tag="op")
                        for h in range(H):
                            nc.tensor.matmul(
                                ps[:], OC[h][:, P * m:P * m + P],
                                wosb[h][:, 512 * n:512 * n + 512],
                                start=(h == 0), stop=(h == H - 1))
                        ot = oout.tile([P, 512], F32, tag="ot")
                        nc.scalar.copy(ot[:], ps[:])
                        nc.sync.dma_start(
                            out[P * m:P * m + P, 512 * n:512 * n + 512], ot[:])

    _split_sync_waits(nc)
    return nc


def _prep_host(x, W_qkv, b_qkv, W_o, with_bias):
    """Build per-core input maps (bf16, pre-transposed/permuted)."""
    bf = ml_dtypes.bfloat16
    Wr = W_qkv.reshape(H, 3 * DK, D)
    br = b_qkv.reshape(H, 3 * DK)

    # wqk: [C, 2048]; cols: r<1024 -> q feat (64g+f), else k feat
    W_qk = np.concatenate(
        [Wr[:, 0:64, :].reshape(H * 64, D), Wr[:, 64:128, :].reshape(H * 64, D)],
        axis=0)
    wqk = W_qk.T
    # wv: [C, 1040]; col 65g+f (f<64) = v feat, col 65g+64 = ones indicator
    C = D + 1 if with_bias else D
    wv = np.zeros((C, H * 65), dtype=np.float32)
    for g in range(H):
        wv[:D, 65 * g:65 * g + 64] = Wr[g, 128:, :].T
        if with_bias:
            wv[D, 65 * g:65 * g + 64] = br[g, 128:]
            wv[D, 65 * g + 64] = 1.0
    if with_bias:
        b_qk = np.concatenate([br[:, 0:64].reshape(-1), br[:, 64:128].reshape(-1)])
        wqk = np.concatenate([wqk, b_qk[None, :]], axis=0)
    wqk = np.ascontiguousarray(wqk, dtype=np.float32).astype(bf)
    wv = wv.astype(bf)
    wo = np.ascontiguousarray(W_o.T, dtype=np.float32).astype(bf)

    in_maps = []
    for c in range(N_CORES):
        b, qh = divmod(c, 2)
        xb = x[b].reshape(H, 128, D)
        if qh == 0:
            xp = xb
        else:
            xp = np.concatenate([xb[:, 64:, :], xb[:, :64, :]], axis=1)
        xp = xp.reshape(S, D).T                      # [1024, 2048]
        if with_bias:
            xp = np.concatenate([xp, np.ones((1, S), np.float32)], axis=0)
        in_maps.append({
            "xT": np.ascontiguousarray(xp, dtype=np.float32).astype(bf),
            "wqk": wqk, "wv": wv, "wo": wo,
        })
    return in_maps


def kernel(x, W_qkv, b_qkv, W_o, b_o):
    x = np.asarray(x, dtype=np.float32)
    W_qkv = np.asarray(W_qkv, dtype=np.float32)
    b_qkv = np.asarray(b_qkv, dtype=np.float32)
    W_o = np.asarray(W_o, dtype=np.float32)
    b_o = np.asarray(b_o, dtype=np.float32)

    with_bias = bool(np.any(b_qkv != 0.0))
    if with_bias not in _PROGRAM_CACHE:
        _PROGRAM_CACHE[with_bias] = _build_program(with_bias)
    nc = _PROGRAM_CACHE[with_bias]

    in_maps = _prep_host(x, W_qkv, b_qkv, W_o, with_bias)
    res = run_bass_kernel_spmd(nc, in_maps, list(range(N_CORES)))

    out = np.empty((B, S, D), dtype=np.float32)
    for c in range(N_CORES):
        b, qh = divmod(c, 2)
        oc = res.results[c]["out"]                   # rows q' = 64g + p
        out[b, 1024 * qh:1024 * qh + 1024, :] = (
            oc.reshape(H, 64, D).transpose(1, 0, 2).reshape(1024, D))
    out += b_o
    return out


if __name__ == "__main__":
    rng = np.random.default_rng(0)
    inputs = {
        "x": rng.standard_normal((B, S, D)).astype(np.float32),
        "W_qkv": (rng.standard_normal((3 * DK * H, D)) * 0.04).astype(np.float32),
        "b_qkv": np.zeros((3 * DK * H,), np.float32),
        "W_o": (rng.standard_normal((DK * H, DK * H)) * 0.03).astype(np.float32),
        "b_o": np.zeros((DK * H,), np.float32),
    }
    o = kernel(**inputs)
    print("kernel out", o.shape, o.dtype)
